# revision 30
# baseline (speedup 1.0000x reference)
"""ContextAwareAttention Trainium2 kernel (v7).

Problem (hardcoded shapes): B=4, S=4096, DIM=256.
  q/k/v = complex linear projections of (z_real, z_imag); q gated by
  sigmoid(context @ wc.T + bc); scores = qf @ kf.T / 16; softmax;
  out = [attn @ v_r, attn @ v_i].

Sharding: 8 cores = 4 batches x 2 query-halves (2048 q rows each).
Host rolls z along the sequence axis per core so the kernel's q rows are
always rows 0..2047 (key-order permutation is softmax-invariant).

v7 design notes:
- Host pre-casts to bf16, pre-transposes, and lays every input out as
  the exact SBUF partition image [128, ...] so DMA descriptors carry
  large contiguous segments (small-segment loads are descriptor-bound
  at ~40ns/descriptor).
- k-projection eliminated algebraically: with p = gated q split into
  (p_r, p_i), scores = u_r . z_r^T + u_i . z_i^T where
  u_r = p_r Wkr + p_i Wki, u_i = p_i Wkr - p_r Wki (q rows only).
- v-projection eliminated the same way on the output side: the AV step
  accumulates out1 = z^T e (same matmul count as attn @ v), and a tiny
  per-q-block post-projection out = out1^T [Wv...] replaces projecting
  v over all 4096 keys: 64 matmuls instead of 128.
- Softmax denominators: DVE accumulates E_sum += e per key-chunk; one
  4-matmul partition-reduction per q-block.
- Projection chunks are interleaved with the attention q-block loops
  (chunk i right before q-block i) so the kernel needs only ~2.8MB of
  input before compute starts; the rest streams in under compute.
- q-block tails (denominators + post-projection + normalize + store)
  are deferred into the next q-block's key loop; out1 PSUM banks are
  freed via copies to SBUF.  One shared 3-buffer PSUM pool serves
  projection psums, score psums, and post-projection psums.
- A matmul warmup burst on zeroed SBUF runs during the DMA preamble so
  the PE's HAM clock-gate is at 8/8 when real work arrives.
- Output is written bf16 as a [128, 16, 512] partition image (pairs of
  row-blocks merged per DMA); host unpacks/casts to the f32 result.
"""

import numpy as np
import ml_dtypes

import concourse.bass as bass
import concourse.mybir as mybir
import concourse.tile as tile
from concourse import bacc, bass_utils

F32 = mybir.dt.float32
BF16 = mybir.dt.bfloat16
BF16NP = ml_dtypes.bfloat16

B, S, D = 4, 4096, 256
D2 = 2 * D          # 512
SQ = S // 2         # 2048 q rows per core
SCALE = D ** (-0.5)
KC = S // 128       # 32 key chunks
QB = SQ // 512      # 4 q blocks of 512


def _build():
    nc = bacc.Bacc("TRN2")
    # All inputs are host-prepped bf16 SBUF partition images.
    z_img = nc.dram_tensor("z_img", [128, 4, S], BF16, kind="ExternalInput")
    zr_img = nc.dram_tensor("zr_img", [128, KC, D2], BF16,
                            kind="ExternalInput")
    ctx_img = nc.dram_tensor("ctx_img", [128, 4, SQ], BF16,
                             kind="ExternalInput")
    wq_img = nc.dram_tensor("wq_img", [128, 3, 2, D], BF16,
                            kind="ExternalInput")
    wk_img = nc.dram_tensor("wk_img", [128, 3, 2, D], BF16,
                            kind="ExternalInput")
    wv_img = nc.dram_tensor("wv_img", [128, 2, 2, D2], BF16,
                            kind="ExternalInput")
    wc_img = nc.dram_tensor("wc_img", [128, 4, D2], BF16,
                            kind="ExternalInput")
    bc_img = nc.dram_tensor("bc_img", [128, 4], F32, kind="ExternalInput")
    out = nc.dram_tensor("out", [128, 16, D2], BF16, kind="ExternalOutput")

    mm = nc.tensor.matmul
    QR, QI, QIN = 0, 1, 2    # wq_img slots
    KR, KI, KIN = 0, 1, 2    # wk_img slots
    VZR, VZI = 0, 1          # wv_img slots

    with tile.TileContext(nc) as tc:
        with tc.tile_pool(name="singles", bufs=1) as singles:
            ones = singles.tile([128, 1], BF16, tag="ones")
            nc.vector.memset(ones, 1.0)

            zT = singles.tile([128, 4, S], BF16, tag="zT")
            zR = singles.tile([128, KC, D2], BF16, tag="zR")
            ctxT = singles.tile([128, 4, SQ], BF16, tag="ctxT")
            qTg = singles.tile([128, 4, SQ], BF16, tag="qTg")
            uT = singles.tile([128, 4, SQ], BF16, tag="uT")

            # chunk-0 inputs first so the PE starts ASAP; the rest
            # streams in underneath compute.
            wc = singles.tile([128, 4, D2], BF16, tag="wc")
            nc.sync.dma_start(out=wc, in_=wc_img[:])
            bcT = singles.tile([128, 4], F32, tag="bcT")
            nc.sync.dma_start(out=bcT, in_=bc_img[:])
            for di in range(4):
                nc.sync.dma_start(
                    out=ctxT[:, di, 0:512], in_=ctx_img[:, di, 0:512])
                nc.sync.dma_start(
                    out=zT[:, di, 0:512], in_=z_img[:, di, 0:512])
            wq = singles.tile([128, 3, 2, D], BF16, tag="wq")
            nc.sync.dma_start(out=wq, in_=wq_img[:])
            wk = singles.tile([128, 3, 2, D], BF16, tag="wk")
            nc.sync.dma_start(out=wk, in_=wk_img[:])
            wv = singles.tile([128, 2, 2, D2], BF16, tag="wv")
            nc.sync.dma_start(out=wv, in_=wv_img[:])
            nc.sync.dma_start(out=zR[:, 0:8, :], in_=zr_img[:, 0:8, :])
            for di in range(4):
                nc.sync.dma_start(
                    out=zT[:, di, 512:1024], in_=z_img[:, di, 512:1024])
                nc.sync.dma_start(
                    out=ctxT[:, di, 512:2048], in_=ctx_img[:, di, 512:2048])
            for di in range(4):
                nc.sync.dma_start(
                    out=zT[:, di, 1024:2048], in_=z_img[:, di, 1024:2048])
            nc.sync.dma_start(out=zR[:, 8:16, :], in_=zr_img[:, 8:16, :])
            for di in range(4):
                nc.sync.dma_start(
                    out=zT[:, di, 2048:3072], in_=z_img[:, di, 2048:3072])
            nc.sync.dma_start(out=zR[:, 16:24, :], in_=zr_img[:, 16:24, :])
            for di in range(4):
                nc.sync.dma_start(
                    out=zT[:, di, 3072:4096], in_=z_img[:, di, 3072:4096])
            nc.sync.dma_start(out=zR[:, 24:32, :], in_=zr_img[:, 24:32, :])

            with (
                tc.tile_pool(name="gsb", bufs=2) as gsb,
                tc.tile_pool(name="esb", bufs=3) as esb,
                tc.tile_pool(name="osb", bufs=2) as osb,
                tc.tile_pool(name="rcp", bufs=2) as rcp,
                tc.tile_pool(name="esum", bufs=2) as esump,
                tc.tile_pool(name="o1sb", bufs=8) as o1sb,
                tc.tile_pool(name="sps", bufs=4, space="PSUM") as sps,
                tc.tile_pool(name="o1p", bufs=4, space="PSUM") as o1p,
            ):
                # HAM warmup: harmless matmuls on zeroed SBUF while the
                # input DMAs land, so real matmuls start at full clock.
                wu = singles.tile([128, 512], BF16, tag="wu")
                nc.vector.memset(wu, 0)
                for _ in range(32):
                    wup = sps.tile([128, 512], F32, tag="sp")
                    mm(wup, wu[:, 0:128], wu, start=True, stop=True)

                def emit_chunk(sc):
                    """gate/q/u projections for q rows sc*512..+512."""
                    r0 = sc * 512
                    for j in range(4):
                        gp = sps.tile([128, 512], F32, tag="sp")
                        for di in range(4):
                            mm(gp, wc[:, di, j * 128:(j + 1) * 128],
                               ctxT[:, di, r0:r0 + 512], start=(di == 0),
                               stop=(di == 3))
                        gate = gsb.tile([128, 512], F32, tag="gate")
                        nc.scalar.activation(
                            out=gate, in_=gp,
                            func=mybir.ActivationFunctionType.Sigmoid,
                            bias=bcT[:, j:j + 1], scale=1.0)
                        qp = sps.tile([128, 512], F32, tag="sp")
                        jj = j % 2
                        if j < 2:   # q_r^T = Wqr z_r^T - Wqi z_i^T
                            terms = [(QR, 0), (QIN, 2)]
                        else:       # q_i^T = Wqr z_i^T + Wqi z_r^T
                            terms = [(QR, 2), (QI, 0)]
                        n = 0
                        for widx, zoff in terms:
                            for dd in range(2):
                                mm(qp,
                                   wq[:, widx, dd, jj * 128:(jj + 1) * 128],
                                   zT[:, zoff + dd, r0:r0 + 512],
                                   start=(n == 0), stop=(n == 3))
                                n += 1
                        nc.vector.tensor_mul(
                            out=qTg[:, j, r0:r0 + 512], in0=qp, in1=gate)

                    # u^T for this q chunk (folds Wk into q side):
                    #   u_r = p_r Wkr + p_i Wki ; u_i = p_i Wkr - p_r Wki
                    for di in range(4):
                        up = sps.tile([128, 512], F32, tag="sp")
                        jj = di % 2
                        if di < 2:
                            terms = [(KR, 0), (KI, 2)]
                        else:
                            terms = [(KR, 2), (KIN, 0)]
                        n = 0
                        for widx, qoff in terms:
                            for a in range(2):
                                mm(up,
                                   wk[:, widx, a, jj * 128:(jj + 1) * 128],
                                   qTg[:, qoff + a, r0:r0 + 512],
                                   start=(n == 0), stop=(n == 3))
                                n += 1
                        nc.vector.tensor_copy(
                            out=uT[:, di, r0:r0 + 512], in_=up)

                def emit_tail(qb, o1s, esb16):
                    # softmax denominators: one partition-reduction per qb.
                    # start=True clears has_written bank-wide, so only the
                    # first matmul starts; later columns' first writes rely
                    # on cleared bits (overwrite+set).
                    sm_t = sps.tile([128, 512], F32, tag="sp", name="sm_t")
                    sm = sm_t[:, 0:4]
                    for qt in range(4):
                        mm(sm[:, qt:qt + 1],
                           esb16[:, qt * 128:(qt + 1) * 128], ones,
                           start=(qt == 0), stop=True)
                    r = rcp.tile([128, 4], F32, tag="r")
                    nc.vector.reciprocal(out=r, in_=sm)
                    # post-projection: out rows = out1^T [Wv combined],
                    # then normalize by 1/rowsum and store.
                    # out image slot m = qb*4 + 2*(qt%2) + qt//2 pairs the
                    # two DVE-normalized blocks (and the two ACT ones) into
                    # adjacent slots -> one 2KB-per-partition DMA each.
                    o_ev = osb.tile([128, 2, D2], BF16, tag="o_ev")
                    o_od = osb.tile([128, 2, D2], BF16, tag="o_od")
                    for qt in range(4):
                        prj = sps.tile([128, 512], F32, tag="sp")
                        n = 0
                        for dd in range(2):
                            mm(prj, o1s[dd][:, qt * 128:(qt + 1) * 128],
                               wv[:, VZR, dd, :], start=(n == 0),
                               stop=(n == 3))
                            n += 1
                            mm(prj, o1s[2 + dd][:, qt * 128:(qt + 1) * 128],
                               wv[:, VZI, dd, :], start=(n == 0),
                               stop=(n == 3))
                            n += 1
                        dst = (o_ev if qt % 2 == 0 else o_od)[:, qt // 2, :]
                        if qt % 2 == 0:
                            nc.vector.tensor_scalar_mul(
                                out=dst, in0=prj, scalar1=r[:, qt:qt + 1])
                        else:
                            nc.scalar.activation(
                                out=dst, in_=prj,
                                func=mybir.ActivationFunctionType.Copy,
                                scale=r[:, qt:qt + 1])
                    nc.sync.dma_start(
                        out=out[:, qb * 4:qb * 4 + 2, :], in_=o_ev)
                    nc.scalar.dma_start(
                        out=out[:, qb * 4 + 2:qb * 4 + 4, :], in_=o_od)

                pending = None
                for qb in range(QB):
                    emit_chunk(qb)
                    out1 = [o1p.tile([128, D2], F32, tag="out1", name="out1")
                            for _ in range(4)]
                    es = esump.tile([128, 512], F32, tag="es")
                    es2 = esump.tile([128, 512], F32, tag="es2")
                    esb16 = esump.tile([128, 512], BF16, tag="esb16")
                    e_prev = None
                    for kc in range(KC):
                        sp = sps.tile([128, 512], F32, tag="sp")
                        for di in range(4):
                            mm(sp, zT[:, di, kc * 128:(kc + 1) * 128],
                               uT[:, di, qb * 512:(qb + 1) * 512],
                               start=(di == 0), stop=(di == 3))
                        e = esb.tile([128, 512], BF16, tag="e")
                        nc.scalar.activation(
                            out=e, in_=sp,
                            func=mybir.ActivationFunctionType.Exp,
                            scale=float(SCALE))
                        # out1 matmuls run one iteration behind the scores
                        # so they never wait on a freshly-computed exp.
                        if e_prev is not None:
                            for dc in range(4):
                                mm(out1[dc],
                                   zR[:, kc - 1, dc * 128:(dc + 1) * 128],
                                   e_prev, start=(kc == 1), stop=False)
                        e_prev = e
                        # two alternating accumulators double the slack in
                        # the serial DVE add chain; merged at the end.
                        acc = es if kc % 2 == 0 else es2
                        if kc < 2:
                            nc.vector.tensor_copy(out=acc, in_=e)
                        elif kc == KC - 1:
                            nc.vector.tensor_add(out=es2, in0=es2, in1=e)
                            if qb < QB - 1:
                                # merge emits the bf16 copy for the matmul
                                nc.vector.tensor_add(
                                    out=esb16, in0=es, in1=es2)
                        else:
                            nc.vector.tensor_add(out=acc, in0=acc, in1=e)
                        if kc == 2 and pending is not None:
                            emit_tail(*pending)
                    for dc in range(4):
                        mm(out1[dc], zR[:, KC - 1, dc * 128:(dc + 1) * 128],
                           e_prev, start=False, stop=True)
                    # free out1 PSUM banks: copy to SBUF (bf16) for the
                    # post-projection, split across DVE and ACT.
                    o1s = []
                    for dc in range(4):
                        a = o1sb.tile([128, D2], BF16, tag="o1s")
                        if dc % 2 == 0:
                            nc.vector.tensor_copy(out=a, in_=out1[dc])
                        else:
                            nc.scalar.activation(
                                out=a, in_=out1[dc],
                                func=mybir.ActivationFunctionType.Copy)
                        o1s.append(a)
                    if qb == QB - 1:
                        # deferred merge: copies went first so the final
                        # post-projection starts as early as possible
                        nc.vector.tensor_add(out=esb16, in0=es, in1=es2)
                    pending = (qb, o1s, esb16)
                emit_tail(*pending)

    nc.finalize()
    return nc


_NC_CACHE = {}


def _img(m):
    """[X*128, Y] f32 -> bf16 SBUF partition image [128, X, Y]."""
    x, y = m.shape
    return np.ascontiguousarray(
        m.reshape(x // 128, 128, y).transpose(1, 0, 2)).astype(BF16NP)


def kernel(z_real, z_imag, context, wq_r, wq_i, wk_r, wk_i, wv_r, wv_i,
           wc, bc, _trace=False, _mm_dt=None):
    if "v7" not in _NC_CACHE:
        _NC_CACHE["v7"] = _build()
    nc = _NC_CACHE["v7"]

    z_real = np.asarray(z_real, dtype=np.float32)
    z_imag = np.asarray(z_imag, dtype=np.float32)
    context = np.asarray(context, dtype=np.float32)
    f32 = lambda x: np.ascontiguousarray(np.asarray(x, dtype=np.float32))
    wq_r, wq_i = f32(wq_r), f32(wq_i)
    wk_r, wk_i = f32(wk_r), f32(wk_i)
    wv_r, wv_i = f32(wv_r), f32(wv_i)
    wc_, bc_ = f32(wc), f32(bc)

    ws = {
        "wq_img": np.stack(
            [_img(wq_r.T), _img(wq_i.T), _img(-wq_i.T)], axis=1),
        "wk_img": np.stack(
            [_img(wk_r), _img(wk_i), _img(-wk_i)], axis=1),
        "wv_img": np.stack(
            [_img(np.ascontiguousarray(np.concatenate(
                [wv_r.T, wv_i.T], axis=1))),
             _img(np.ascontiguousarray(np.concatenate(
                 [-wv_i.T, wv_r.T], axis=1)))], axis=1),
        "wc_img": _img(np.ascontiguousarray(wc_.T)),
        "bc_img": np.ascontiguousarray(
            bc_.reshape(4, 128).T),
    }

    in_maps = []
    for c in range(8):
        b, h = c // 2, c % 2
        zr = np.roll(z_real[b], -h * SQ, axis=0)
        zi = np.roll(z_imag[b], -h * SQ, axis=0)
        cx = context[b, h * SQ:(h + 1) * SQ]
        in_maps.append({
            "z_img": np.concatenate(
                [_img(np.ascontiguousarray(zr.T)),
                 _img(np.ascontiguousarray(zi.T))], axis=1),
            "zr_img": _img(np.concatenate([zr, zi], axis=1)),
            "ctx_img": _img(np.ascontiguousarray(cx.T)),
            **ws,
        })
    res = bass_utils.run_bass_kernel_spmd(
        nc, in_maps, core_ids=list(range(8)), trace=_trace)

    # unpack the out image: slot m = qb*4 + 2*(qt%2) + qt//2
    full = np.empty((B, S, D2), dtype=np.float32)
    for c in range(8):
        b, h = c // 2, c % 2
        img = np.asarray(res.results[c]["out"], dtype=np.float32)
        dst = full[b, h * SQ:(h + 1) * SQ, :]
        for m in range(16):
            qb, u = divmod(m, 4)
            qt = 2 * (u % 2) + u // 2
            i = qb * 4 + qt
            dst[i * 128:(i + 1) * 128, :] = img[:, m, :]
    if _trace:
        return full, res
    return full


# revision 31
# speedup vs baseline: 1.0061x; 1.0061x over previous
"""ContextAwareAttention Trainium2 kernel (v7).

Problem (hardcoded shapes): B=4, S=4096, DIM=256.
  q/k/v = complex linear projections of (z_real, z_imag); q gated by
  sigmoid(context @ wc.T + bc); scores = qf @ kf.T / 16; softmax;
  out = [attn @ v_r, attn @ v_i].

Sharding: 8 cores = 4 batches x 2 query-halves (2048 q rows each).
Host rolls z along the sequence axis per core so the kernel's q rows are
always rows 0..2047 (key-order permutation is softmax-invariant).

v7 design notes:
- Host pre-casts to bf16, pre-transposes, and lays every input out as
  the exact SBUF partition image [128, ...] so DMA descriptors carry
  large contiguous segments (small-segment loads are descriptor-bound
  at ~40ns/descriptor).
- k-projection eliminated algebraically: with p = gated q split into
  (p_r, p_i), scores = u_r . z_r^T + u_i . z_i^T where
  u_r = p_r Wkr + p_i Wki, u_i = p_i Wkr - p_r Wki (q rows only).
- v-projection eliminated the same way on the output side: the AV step
  accumulates out1 = z^T e (same matmul count as attn @ v), and a tiny
  per-q-block post-projection out = out1^T [Wv...] replaces projecting
  v over all 4096 keys: 64 matmuls instead of 128.
- Softmax denominators: DVE accumulates E_sum += e per key-chunk; one
  4-matmul partition-reduction per q-block.
- Projection chunks are interleaved with the attention q-block loops
  (chunk i right before q-block i) so the kernel needs only ~2.8MB of
  input before compute starts; the rest streams in under compute.
- q-block tails (denominators + post-projection + normalize + store)
  are deferred into the next q-block's key loop; out1 PSUM banks are
  freed via copies to SBUF.  One shared 3-buffer PSUM pool serves
  projection psums, score psums, and post-projection psums.
- A matmul warmup burst on zeroed SBUF runs during the DMA preamble so
  the PE's HAM clock-gate is at 8/8 when real work arrives.
- Output is written bf16 as a [128, 16, 512] partition image (pairs of
  row-blocks merged per DMA); host unpacks/casts to the f32 result.
"""

import numpy as np
import ml_dtypes

import concourse.bass as bass
import concourse.mybir as mybir
import concourse.tile as tile
from concourse import bacc, bass_utils

F32 = mybir.dt.float32
BF16 = mybir.dt.bfloat16
BF16NP = ml_dtypes.bfloat16

B, S, D = 4, 4096, 256
D2 = 2 * D          # 512
SQ = S // 2         # 2048 q rows per core
SCALE = D ** (-0.5)
KC = S // 128       # 32 key chunks
QB = SQ // 512      # 4 q blocks of 512


def _build():
    nc = bacc.Bacc("TRN2")
    # All inputs are host-prepped bf16 SBUF partition images.
    z_img = nc.dram_tensor("z_img", [128, 4, S], BF16, kind="ExternalInput")
    zr_img = nc.dram_tensor("zr_img", [128, KC, D2], BF16,
                            kind="ExternalInput")
    ctx_img = nc.dram_tensor("ctx_img", [128, 4, SQ], BF16,
                             kind="ExternalInput")
    wq_img = nc.dram_tensor("wq_img", [128, 3, 2, D], BF16,
                            kind="ExternalInput")
    wk_img = nc.dram_tensor("wk_img", [128, 3, 2, D], BF16,
                            kind="ExternalInput")
    wv_img = nc.dram_tensor("wv_img", [128, 2, 2, D2], BF16,
                            kind="ExternalInput")
    wc_img = nc.dram_tensor("wc_img", [128, 4, D2], BF16,
                            kind="ExternalInput")
    bc_img = nc.dram_tensor("bc_img", [128, 4], F32, kind="ExternalInput")
    out = nc.dram_tensor("out", [128, 16, D2], BF16, kind="ExternalOutput")

    mm = nc.tensor.matmul
    QR, QI, QIN = 0, 1, 2    # wq_img slots
    KR, KI, KIN = 0, 1, 2    # wk_img slots
    VZR, VZI = 0, 1          # wv_img slots

    with tile.TileContext(nc) as tc:
        with tc.tile_pool(name="singles", bufs=1) as singles:
            ones = singles.tile([128, 1], BF16, tag="ones")
            nc.vector.memset(ones, 1.0)

            zT = singles.tile([128, 4, S], BF16, tag="zT")
            zR = singles.tile([128, KC, D2], BF16, tag="zR")
            ctxT = singles.tile([128, 4, SQ], BF16, tag="ctxT")
            qTg = singles.tile([128, 4, SQ], BF16, tag="qTg")
            uT = singles.tile([128, 4, SQ], BF16, tag="uT")

            # chunk-0 inputs first so the PE starts ASAP; the rest
            # streams in underneath compute.
            wc = singles.tile([128, 4, D2], BF16, tag="wc")
            nc.sync.dma_start(out=wc, in_=wc_img[:])
            bcT = singles.tile([128, 4], F32, tag="bcT")
            nc.sync.dma_start(out=bcT, in_=bc_img[:])
            for di in range(4):
                nc.sync.dma_start(
                    out=ctxT[:, di, 0:512], in_=ctx_img[:, di, 0:512])
                nc.sync.dma_start(
                    out=zT[:, di, 0:512], in_=z_img[:, di, 0:512])
            wq = singles.tile([128, 3, 2, D], BF16, tag="wq")
            nc.sync.dma_start(out=wq, in_=wq_img[:])
            wk = singles.tile([128, 3, 2, D], BF16, tag="wk")
            nc.sync.dma_start(out=wk, in_=wk_img[:])
            wv = singles.tile([128, 2, 2, D2], BF16, tag="wv")
            nc.sync.dma_start(out=wv, in_=wv_img[:])
            nc.sync.dma_start(out=zR[:, 0:8, :], in_=zr_img[:, 0:8, :])
            for di in range(4):
                nc.sync.dma_start(
                    out=zT[:, di, 512:1024], in_=z_img[:, di, 512:1024])
                nc.sync.dma_start(
                    out=ctxT[:, di, 512:2048], in_=ctx_img[:, di, 512:2048])
            for di in range(4):
                nc.sync.dma_start(
                    out=zT[:, di, 1024:2048], in_=z_img[:, di, 1024:2048])
            nc.sync.dma_start(out=zR[:, 8:16, :], in_=zr_img[:, 8:16, :])
            for di in range(4):
                nc.sync.dma_start(
                    out=zT[:, di, 2048:3072], in_=z_img[:, di, 2048:3072])
            nc.sync.dma_start(out=zR[:, 16:24, :], in_=zr_img[:, 16:24, :])
            for di in range(4):
                nc.sync.dma_start(
                    out=zT[:, di, 3072:4096], in_=z_img[:, di, 3072:4096])
            nc.sync.dma_start(out=zR[:, 24:32, :], in_=zr_img[:, 24:32, :])

            with (
                tc.tile_pool(name="gsb", bufs=2) as gsb,
                tc.tile_pool(name="esb", bufs=3) as esb,
                tc.tile_pool(name="osb", bufs=2) as osb,
                tc.tile_pool(name="rcp", bufs=2) as rcp,
                tc.tile_pool(name="esum", bufs=2) as esump,
                tc.tile_pool(name="o1sb", bufs=8) as o1sb,
                tc.tile_pool(name="sps", bufs=3, space="PSUM") as sps,
                tc.tile_pool(name="o1p", bufs=4, space="PSUM") as o1p,
                tc.tile_pool(name="smp", bufs=1, space="PSUM") as smp,
            ):
                # HAM warmup: harmless matmuls on zeroed SBUF while the
                # input DMAs land, so real matmuls start at full clock.
                wu = singles.tile([128, 512], BF16, tag="wu")
                nc.vector.memset(wu, 0)
                for _ in range(32):
                    wup = sps.tile([128, 512], F32, tag="sp")
                    mm(wup, wu[:, 0:128], wu, start=True, stop=True)

                def emit_chunk(sc):
                    """gate/q/u projections for q rows sc*512..+512."""
                    r0 = sc * 512
                    for j in range(4):
                        gp = sps.tile([128, 512], F32, tag="sp")
                        for di in range(4):
                            mm(gp, wc[:, di, j * 128:(j + 1) * 128],
                               ctxT[:, di, r0:r0 + 512], start=(di == 0),
                               stop=(di == 3))
                        gate = gsb.tile([128, 512], F32, tag="gate")
                        nc.scalar.activation(
                            out=gate, in_=gp,
                            func=mybir.ActivationFunctionType.Sigmoid,
                            bias=bcT[:, j:j + 1], scale=1.0)
                        qp = sps.tile([128, 512], F32, tag="sp")
                        jj = j % 2
                        if j < 2:   # q_r^T = Wqr z_r^T - Wqi z_i^T
                            terms = [(QR, 0), (QIN, 2)]
                        else:       # q_i^T = Wqr z_i^T + Wqi z_r^T
                            terms = [(QR, 2), (QI, 0)]
                        n = 0
                        for widx, zoff in terms:
                            for dd in range(2):
                                mm(qp,
                                   wq[:, widx, dd, jj * 128:(jj + 1) * 128],
                                   zT[:, zoff + dd, r0:r0 + 512],
                                   start=(n == 0), stop=(n == 3))
                                n += 1
                        nc.vector.tensor_mul(
                            out=qTg[:, j, r0:r0 + 512], in0=qp, in1=gate)

                    # u^T for this q chunk (folds Wk into q side):
                    #   u_r = p_r Wkr + p_i Wki ; u_i = p_i Wkr - p_r Wki
                    for di in range(4):
                        up = sps.tile([128, 512], F32, tag="sp")
                        jj = di % 2
                        if di < 2:
                            terms = [(KR, 0), (KI, 2)]
                        else:
                            terms = [(KR, 2), (KIN, 0)]
                        n = 0
                        for widx, qoff in terms:
                            for a in range(2):
                                mm(up,
                                   wk[:, widx, a, jj * 128:(jj + 1) * 128],
                                   qTg[:, qoff + a, r0:r0 + 512],
                                   start=(n == 0), stop=(n == 3))
                                n += 1
                        nc.vector.tensor_copy(
                            out=uT[:, di, r0:r0 + 512], in_=up)

                def emit_tail(qb, o1s, esb16):
                    # softmax denominators: one partition-reduction per qb.
                    # start=True clears has_written bank-wide, so only the
                    # first matmul starts; later columns' first writes rely
                    # on cleared bits (overwrite+set).
                    sm = smp.tile([128, 4], F32, tag="sm")
                    for qt in range(4):
                        mm(sm[:, qt:qt + 1],
                           esb16[:, qt * 128:(qt + 1) * 128], ones,
                           start=(qt == 0), stop=True)
                    r = rcp.tile([128, 4], F32, tag="r")
                    nc.vector.reciprocal(out=r, in_=sm)
                    # post-projection: out rows = out1^T [Wv combined],
                    # then normalize by 1/rowsum and store.
                    # out image slot m = qb*4 + 2*(qt%2) + qt//2 pairs the
                    # two DVE-normalized blocks (and the two ACT ones) into
                    # adjacent slots -> one 2KB-per-partition DMA each.
                    o_ev = osb.tile([128, 2, D2], BF16, tag="o_ev")
                    o_od = osb.tile([128, 2, D2], BF16, tag="o_od")
                    for qt in range(4):
                        prj = sps.tile([128, 512], F32, tag="sp")
                        n = 0
                        for dd in range(2):
                            mm(prj, o1s[dd][:, qt * 128:(qt + 1) * 128],
                               wv[:, VZR, dd, :], start=(n == 0),
                               stop=(n == 3))
                            n += 1
                            mm(prj, o1s[2 + dd][:, qt * 128:(qt + 1) * 128],
                               wv[:, VZI, dd, :], start=(n == 0),
                               stop=(n == 3))
                            n += 1
                        dst = (o_ev if qt % 2 == 0 else o_od)[:, qt // 2, :]
                        if qt % 2 == 0:
                            nc.vector.tensor_scalar_mul(
                                out=dst, in0=prj, scalar1=r[:, qt:qt + 1])
                        else:
                            nc.scalar.activation(
                                out=dst, in_=prj,
                                func=mybir.ActivationFunctionType.Copy,
                                scale=r[:, qt:qt + 1])
                    nc.sync.dma_start(
                        out=out[:, qb * 4:qb * 4 + 2, :], in_=o_ev)
                    nc.scalar.dma_start(
                        out=out[:, qb * 4 + 2:qb * 4 + 4, :], in_=o_od)

                pending = None
                for qb in range(QB):
                    emit_chunk(qb)
                    out1 = [o1p.tile([128, D2], F32, tag="out1", name="out1")
                            for _ in range(4)]
                    es = esump.tile([128, 512], F32, tag="es")
                    es2 = esump.tile([128, 512], F32, tag="es2")
                    esb16 = esump.tile([128, 512], BF16, tag="esb16")
                    e_prev = None
                    for kc in range(KC):
                        sp = sps.tile([128, 512], F32, tag="sp")
                        for di in range(4):
                            mm(sp, zT[:, di, kc * 128:(kc + 1) * 128],
                               uT[:, di, qb * 512:(qb + 1) * 512],
                               start=(di == 0), stop=(di == 3))
                        e = esb.tile([128, 512], BF16, tag="e")
                        nc.scalar.activation(
                            out=e, in_=sp,
                            func=mybir.ActivationFunctionType.Exp,
                            scale=float(SCALE))
                        # out1 matmuls run one iteration behind the scores
                        # so they never wait on a freshly-computed exp.
                        if e_prev is not None:
                            for dc in range(4):
                                mm(out1[dc],
                                   zR[:, kc - 1, dc * 128:(dc + 1) * 128],
                                   e_prev, start=(kc == 1), stop=False)
                        e_prev = e
                        # two alternating accumulators double the slack in
                        # the serial DVE add chain; merged at the end.
                        acc = es if kc % 2 == 0 else es2
                        if kc < 2:
                            nc.vector.tensor_copy(out=acc, in_=e)
                        elif kc == KC - 1:
                            nc.vector.tensor_add(out=es2, in0=es2, in1=e)
                            if qb < QB - 1:
                                # merge emits the bf16 copy for the matmul
                                nc.vector.tensor_add(
                                    out=esb16, in0=es, in1=es2)
                        else:
                            nc.vector.tensor_add(out=acc, in0=acc, in1=e)
                        if kc == 2 and pending is not None:
                            emit_tail(*pending)
                    for dc in range(4):
                        mm(out1[dc], zR[:, KC - 1, dc * 128:(dc + 1) * 128],
                           e_prev, start=False, stop=True)
                    # free out1 PSUM banks: copy to SBUF (bf16) for the
                    # post-projection, split across DVE and ACT.
                    o1s = []
                    for dc in range(4):
                        a = o1sb.tile([128, D2], BF16, tag="o1s")
                        if dc % 2 == 0:
                            nc.vector.tensor_copy(out=a, in_=out1[dc])
                        else:
                            nc.scalar.activation(
                                out=a, in_=out1[dc],
                                func=mybir.ActivationFunctionType.Copy)
                        o1s.append(a)
                    if qb == QB - 1:
                        # deferred merge: copies went first so the final
                        # post-projection starts as early as possible
                        nc.vector.tensor_add(out=esb16, in0=es, in1=es2)
                    pending = (qb, o1s, esb16)
                emit_tail(*pending)

    nc.finalize()
    return nc


_NC_CACHE = {}


def _img(m):
    """[X*128, Y] f32 -> bf16 SBUF partition image [128, X, Y]."""
    x, y = m.shape
    return np.ascontiguousarray(
        m.reshape(x // 128, 128, y).transpose(1, 0, 2)).astype(BF16NP)


def kernel(z_real, z_imag, context, wq_r, wq_i, wk_r, wk_i, wv_r, wv_i,
           wc, bc, _trace=False, _mm_dt=None):
    if "v7" not in _NC_CACHE:
        _NC_CACHE["v7"] = _build()
    nc = _NC_CACHE["v7"]

    z_real = np.asarray(z_real, dtype=np.float32)
    z_imag = np.asarray(z_imag, dtype=np.float32)
    context = np.asarray(context, dtype=np.float32)
    f32 = lambda x: np.ascontiguousarray(np.asarray(x, dtype=np.float32))
    wq_r, wq_i = f32(wq_r), f32(wq_i)
    wk_r, wk_i = f32(wk_r), f32(wk_i)
    wv_r, wv_i = f32(wv_r), f32(wv_i)
    wc_, bc_ = f32(wc), f32(bc)

    ws = {
        "wq_img": np.stack(
            [_img(wq_r.T), _img(wq_i.T), _img(-wq_i.T)], axis=1),
        "wk_img": np.stack(
            [_img(wk_r), _img(wk_i), _img(-wk_i)], axis=1),
        "wv_img": np.stack(
            [_img(np.ascontiguousarray(np.concatenate(
                [wv_r.T, wv_i.T], axis=1))),
             _img(np.ascontiguousarray(np.concatenate(
                 [-wv_i.T, wv_r.T], axis=1)))], axis=1),
        "wc_img": _img(np.ascontiguousarray(wc_.T)),
        "bc_img": np.ascontiguousarray(
            bc_.reshape(4, 128).T),
    }

    in_maps = []
    for c in range(8):
        b, h = c // 2, c % 2
        zr = np.roll(z_real[b], -h * SQ, axis=0)
        zi = np.roll(z_imag[b], -h * SQ, axis=0)
        cx = context[b, h * SQ:(h + 1) * SQ]
        in_maps.append({
            "z_img": np.concatenate(
                [_img(np.ascontiguousarray(zr.T)),
                 _img(np.ascontiguousarray(zi.T))], axis=1),
            "zr_img": _img(np.concatenate([zr, zi], axis=1)),
            "ctx_img": _img(np.ascontiguousarray(cx.T)),
            **ws,
        })
    res = bass_utils.run_bass_kernel_spmd(
        nc, in_maps, core_ids=list(range(8)), trace=_trace)

    # unpack the out image: slot m = qb*4 + 2*(qt%2) + qt//2
    full = np.empty((B, S, D2), dtype=np.float32)
    for c in range(8):
        b, h = c // 2, c % 2
        img = np.asarray(res.results[c]["out"], dtype=np.float32)
        dst = full[b, h * SQ:(h + 1) * SQ, :]
        for m in range(16):
            qb, u = divmod(m, 4)
            qt = 2 * (u % 2) + u // 2
            i = qb * 4 + qt
            dst[i * 128:(i + 1) * 128, :] = img[:, m, :]
    if _trace:
        return full, res
    return full
